# revision 1
# baseline (speedup 1.0000x reference)
"""Trainium2 Bass kernel for a fused-QKV LoRA merged linear.

Reference math (nn_BaseMergedLinear): out = x @ W.T where
W = zero_pad(concat_g(B_g @ A_g)) with blocks [Q, K, V], LoRA enabled on
blocks 0 and 2 only.  Block 1 (K) of the output is identically zero, so the
device only computes the two enabled blocks:

    out_g = (x @ A_g.T) @ B_g.T        g in {0, 1}

Sharding: data-parallel over the 1024 tokens (128 per core, 8 cores).
weight_A / weight_B are replicated.  Host pre-packs weights and the x shard
into PE-friendly layouts (contraction dim on partitions); the zero middle
block is assembled on the host.

Device program per core:
  stage 1: t.T (48p x 128tok PSUM)  = sum_n Apad_chunk.T @ xT_chunk, 32 chunks
  stage 2: out (128tok x 512) psum  = tT[g].T @ Bstack[g], 16 matmuls (N=512),
           groups at PE row-offsets 0/32 so they pack; DVE copy -> SBUF ->
           HWDGE DMA to DRAM.
"""

import numpy as np

import concourse.bass as bass
import concourse.mybir as mybir
from concourse import bacc
from concourse.tile import TileContext, add_dep_helper
from concourse.bass_utils import run_bass_kernel_spmd

N_CORES = 8
TOK = 128              # tokens per core
IN_F = 4096
N_KCH = IN_F // 128    # 32 contraction chunks
R = 16
OUT_PG = 4096          # output cols per enabled group
N_OUT = 2 * OUT_PG     # device output cols per core (enabled blocks only)
FULL_OUT = 12288
AP_M = 48              # padded stage-1 M: group0 rows 0:16, group1 rows 32:48

F32 = mybir.dt.float32

_NC_CACHE = {}


def build_nc(psum_bufs: int = 5, stag_bufs: int = 8,
             n_stores: int = 8, n_warmup: int = 10, s2_warm_every: int = 1):
    """Build the single-core Bass program (same program on all 8 cores)."""
    # Bacc (not plain Bass): its compile() runs generate_event_semaphores,
    # which legalizes multi-wait instructions for TRN2 (1 wait/instruction).
    nc = bacc.Bacc()
    # All 8 cores pull from shared HBM simultaneously, so the load phase runs
    # at the shared roofline — the only levers are starting stage 1 on the
    # smallest early chunks and keeping HBM streaming continuously.  Each
    # tensor is host-packed fully contiguous (long descriptor runs).
    AW = N_KCH * AP_M // 2                  # apad half cols (16 k-chunks)
    apads = [nc.dram_tensor(f"apad{h}", [128, AW], F32, kind="ExternalInput")
             for h in range(2)]
    xts = [nc.dram_tensor(f"xt{i}", [128, IN_F // 4], F32, kind="ExternalInput")
           for i in range(4)]
    b0 = nc.dram_tensor("b0", [R, OUT_PG], F32, kind="ExternalInput")
    b1 = nc.dram_tensor("b1", [R, OUT_PG], F32, kind="ExternalInput")
    out = nc.dram_tensor("out", [TOK, N_OUT], F32, kind="ExternalOutput")

    with TileContext(nc) as tc:
        with (
            tc.tile_pool(name="wpool", bufs=1) as wp,
            tc.tile_pool(name="xpool", bufs=1) as xp,
            tc.tile_pool(name="ps1", bufs=1, space="PSUM") as pp1,
            tc.tile_pool(name="ps2", bufs=psum_bufs, space="PSUM") as pp2,
            tc.tile_pool(name="stag", bufs=stag_bufs) as sp,
        ):
            # PE clock warmup: the HAM throttles the PE to 1.2 GHz until it
            # has been busy ~3.4us.  The PE would otherwise idle during the
            # load phase and run the whole kernel cold (2x slower matmuls).
            # Burn idle time on zero matmuls so real matmuls start warm.
            wz = wp.tile([128, 512], mybir.dt.bfloat16, tag="wz")
            nc.gpsimd.memset(wz[:], 0.0)
            wps = pp1.tile([128, 512], F32, tag="wps")
            for _ in range(n_warmup):
                nc.tensor.matmul(wps[:], lhsT=wz[:, 0:128], rhs=wz[:],
                                 start=True, stop=True)

            # Ring order is dependency order; the PSUM accumulation consumes
            # x chunks in ARRIVAL order (x0, x2, x1, x3) — accumulation is
            # order-agnostic:
            #   sync ring:   [apad_a, apad_b, x1, x3]
            #   scalar ring: [x0, x2, b0, b1]
            a_sbs = []
            for h in range(2):
                atl = xp.tile([128, AW], F32, name=f"a{h}", tag=f"a{h}")
                nc.sync.dma_start(out=atl[:], in_=apads[h][:])
                a_sbs.append(atl)
            x_tiles = []
            for i in range(4):
                xtl = xp.tile([128, IN_F // 4], F32, name=f"x{i}", tag=f"x{i}")
                if i % 2 == 0:
                    nc.scalar.dma_start(out=xtl[:], in_=xts[i][:])
                x_tiles.append(xtl)
            for i in (1, 3):
                nc.sync.dma_start(out=x_tiles[i][:], in_=xts[i][:])
            b_sb = wp.tile([64, OUT_PG], F32, tag="b")
            nc.scalar.dma_start(out=b_sb[0:R, :], in_=b0[:])
            nc.scalar.dma_start(out=b_sb[32:32 + R, :], in_=b1[:])

            # stage 1: accumulate t.T = Apad @ x_core.T over 32 k-chunks,
            # consuming chunks in DMA-arrival order (accumulation commutes)
            tps = pp1.tile([AP_M, TOK], F32)
            order = list(range(0, 8)) + list(range(16, 24)) + \
                list(range(8, 16)) + list(range(24, 32))
            s1_mm = None
            for idx, n in enumerate(order):
                # The initial warm lease expires ~6.8us in (mid-stage-1) and
                # M=48 matmuls sit at the HAM qualification borderline; a
                # pinned fat-filler block right at the expiry point holds
                # 2.4GHz for the second half.
                if idx == 16 and s1_mm is not None:
                    for _ in range(6):
                        wmm = nc.tensor.matmul(wps[:], lhsT=wz[:, 0:128],
                                               rhs=wz[:], start=True, stop=True)
                        add_dep_helper(wmm.ins, s1_mm.ins, sync=False,
                                       reason="pin stage-1 HAM filler")
                s1_mm = nc.tensor.matmul(
                    tps[:],
                    lhsT=a_sbs[n // 16][:, (n % 16) * AP_M:(n % 16 + 1) * AP_M],
                    rhs=x_tiles[n // 8][:, (n % 8) * 128:(n % 8) * 128 + 128],
                    start=(idx == 0),
                    stop=(idx == N_KCH - 1),
                )
            t_sb = wp.tile([AP_M, TOK], F32, tag="t")
            nc.vector.tensor_copy(t_sb[:], tps[:])

            # Tiny 1x1 matmuls absorb the B-DMA semaphore waits so the first
            # stage-2 matmul of each group needs at most one new semaphore.
            warm = pp1.tile([1, 1], F32, tag="warm")
            nc.tensor.matmul(warm[:], lhsT=b_sb[0:1, 0:1], rhs=b_sb[0:1, 0:1],
                             start=True, stop=True)
            nc.tensor.matmul(warm[:], lhsT=b_sb[32:33, 0:1], rhs=b_sb[32:33, 0:1],
                             start=True, stop=True)

            # stage 2: out[tok, o] per group, 512-col psum chunks, staged to
            # SBUF and stored in n_stores big DMAs, each covering the same
            # column range of BOTH groups via a 3D (tok, group, cols) AP.
            #
            # Sem budget: the kernel keeps total DMA instructions small so
            # that (a) no DMAHW lane is reused (a reused lane adds a second
            # sync wait to the DMA, and the HWDGE DMA ISA slot holds only
            # one) and (b) the Tile kernel-tail Drain's wait list (PE + DVE +
            # one per DMA lane) stays within its slot budget.
            per_store = (OUT_PG // 512) // n_stores  # psum chunks/group/store
            stw = per_store * 512                    # cols/group/store
            prev_mm = None
            for s in range(n_stores):
                stg = sp.tile([TOK, 2 * stw], F32, name="stg", tag="stg")
                for j2 in range(per_store):
                    j_abs = s * per_store + j2
                    # keep the HAM warm through stage 2: its K=16 matmuls are
                    # too thin to requalify the warm lease on their own.  Pin
                    # each filler behind the previous real matmul so the
                    # scheduler can't hoist it to the front of the PE queue.
                    # Re-qualify the HAM warm lease mid-stage-2: K=16 matmuls
                    # exercise 12.5% of the array and can never re-warm the
                    # clock themselves.  A solid ~3.4us block of fat bf16
                    # matmuls after pair 2 buys 2.4GHz for the remaining
                    # pairs (cold pair 2.5us vs warm 1.2us).
                    if (s2_warm_every and j_abs in (2, 5)
                            and prev_mm is not None):
                        for _ in range(8 if j_abs == 2 else 4):
                            wmm = nc.tensor.matmul(wps[:], lhsT=wz[:, 0:128],
                                                   rhs=wz[:], start=True,
                                                   stop=True)
                            add_dep_helper(wmm.ins, prev_mm.ins, sync=False,
                                           reason="pin stage-2 HAM filler")
                    for g in (0, 1):
                        j = s * per_store + j2
                        ps = pp2.tile([TOK, 512], F32)
                        prev_mm = nc.tensor.matmul(
                            ps[:],
                            lhsT=t_sb[32 * g:32 * g + 16, :],
                            rhs=b_sb[32 * g:32 * g + 16, j * 512:(j + 1) * 512],
                            start=True,
                            stop=True,
                        )
                        # split the PSUM drain across DVE and ACT: two 690ns
                        # DVE copies per chunk-pair out-pace the warm PE and
                        # stall the matmul pipeline behind PSUM-slot reuse
                        dst_sl = stg[:, (g * per_store + j2) * 512:
                                     (g * per_store + j2 + 1) * 512]
                        if g == 0:
                            nc.vector.tensor_copy(dst_sl, ps[:])
                        else:
                            nc.scalar.copy(dst_sl, ps[:])
                eng = [nc.scalar, nc.gpsimd, nc.sync][s % 3]
                dst = out.rearrange("t (g o) -> t g o", g=2)[:, :, s * stw:(s + 1) * stw]
                src = stg.rearrange("t (g o) -> t g o", g=2)
                eng.dma_start(out=dst, in_=src)
    nc.compile()
    return nc


def prep_weights(weight_A: np.ndarray, weight_B: np.ndarray):
    """Pack weights into the PE layouts (replicated across cores)."""
    weight_A = np.asarray(weight_A, np.float32)
    weight_B = np.asarray(weight_B, np.float32)
    A_pad = np.zeros((AP_M, IN_F), np.float32)
    A_pad[0:16] = weight_A[0:16]      # group 0 (block Q)
    A_pad[32:48] = weight_A[16:32]    # group 1 (block V)
    apad = np.ascontiguousarray(
        A_pad.reshape(AP_M, N_KCH, 128).transpose(2, 1, 0)
    ).reshape(128, N_KCH * AP_M)
    b0 = np.ascontiguousarray(weight_B[0:OUT_PG].T)
    b1 = np.ascontiguousarray(weight_B[OUT_PG:2 * OUT_PG].T)
    return apad, b0, b1


def prep_x_shard(xs: np.ndarray) -> np.ndarray:
    """(128, 4096) token shard -> (128, 4096) transposed-tiled layout where
    tile[p, n*128+t] = xs[t, n*128+p] (contraction dim on partitions)."""
    return np.ascontiguousarray(
        xs.reshape(TOK, N_KCH, 128).transpose(2, 1, 0)
    ).reshape(128, IN_F)


def make_in_maps(x: np.ndarray, weight_A: np.ndarray, weight_B: np.ndarray):
    xs_full = np.asarray(x, np.float32).reshape(N_CORES * TOK, IN_F)
    apad, b0, b1 = prep_weights(weight_A, weight_B)
    AW = N_KCH * AP_M // 2
    apad_h = [np.ascontiguousarray(apad[:, h * AW:(h + 1) * AW]) for h in range(2)]
    in_maps = []
    for c in range(N_CORES):
        xt = prep_x_shard(xs_full[c * TOK:(c + 1) * TOK])
        m = {"apad0": apad_h[0], "apad1": apad_h[1], "b0": b0, "b1": b1}
        for i in range(4):
            m[f"xt{i}"] = np.ascontiguousarray(
                xt[:, i * (IN_F // 4):(i + 1) * (IN_F // 4)])
        in_maps.append(m)
    return in_maps


def assemble_output(results) -> np.ndarray:
    full = np.zeros((N_CORES * TOK, FULL_OUT), np.float32)
    for c in range(N_CORES):
        o = results[c]["out"]
        full[c * TOK:(c + 1) * TOK, 0:OUT_PG] = o[:, 0:OUT_PG]
        full[c * TOK:(c + 1) * TOK, 2 * OUT_PG:3 * OUT_PG] = o[:, OUT_PG:2 * OUT_PG]
    return full.reshape(2, 512, FULL_OUT)


def run(x, weight_A, weight_B, **spmd_kwargs):
    key = "default"
    if key not in _NC_CACHE:
        _NC_CACHE[key] = build_nc()
    nc = _NC_CACHE[key]
    in_maps = make_in_maps(x, weight_A, weight_B)
    res = run_bass_kernel_spmd(nc, in_maps, list(range(N_CORES)), **spmd_kwargs)
    return assemble_output(res.results), res


def kernel(x, weight_A, weight_B):
    out, _ = run(x, weight_A, weight_B)
    return out



# revision 4
# speedup vs baseline: 1.5584x; 1.5584x over previous
"""Trainium2 Bass kernel for a fused-QKV LoRA merged linear.

Reference math (nn_BaseMergedLinear): out = x @ W.T where
W = zero_pad(concat_g(B_g @ A_g)) with blocks [Q, K, V], LoRA enabled on
blocks 0 and 2 only.  Block 1 (K) of the output is identically zero, so the
device only computes the two enabled blocks:

    out_g = (x @ A_g.T) @ B_g.T        g in {0, 1}

Sharding: data-parallel over the 1024 tokens (128 per core, 8 cores).
weight_A / weight_B are replicated.  All device I/O is bf16 (the 2e-2
rel-err budget dwarfs bf16's ~2.5e-3): halves HBM traffic vs f32.

Device program per core:
  stage 1: t (48p x 128tok PSUM f32) accumulated over 32 k-chunks as
           col-tiled concurrent MM pairs (g0 -> psum rows 0:16 via
           tile_position (0,0), g1 -> rows 32:48 via (0,32)).
  stage 2: per 512-col chunk, row-tiled concurrent MM pair
           (t[0:16]/t[32:48] x B chunks) -> PSUM f32, cast-copied to bf16
           staging (DVE/ACT/POOL round-robin), stored in 8 pipelined
           256KB DMAs, each covering both groups via a 3D (tok,g,col) AP.
"""

import numpy as np
import ml_dtypes

import concourse.bass as bass
import concourse.mybir as mybir
from concourse import bacc
from concourse.tile import TileContext, add_dep_helper
from concourse.bass_utils import run_bass_kernel_spmd

N_CORES = 8
TOK = 128              # tokens per core
IN_F = 4096
N_KCH = IN_F // 128    # 32 contraction chunks
R = 16
OUT_PG = 4096          # output cols per enabled group
N_OUT = 2 * OUT_PG     # device output cols per core (enabled blocks only)
FULL_OUT = 12288

F32 = mybir.dt.float32
BF16 = mybir.dt.bfloat16
NPBF16 = ml_dtypes.bfloat16

_NC_CACHE = {}


def build_nc(psum_bufs: int = 6, stag_bufs: int = 8,
             n_stores: int = 8, n_warmup: int = 12):
    """Build the single-core Bass program (same program on all 8 cores)."""
    nc = bacc.Bacc()
    # bf16 inputs, host-packed contiguous.
    a_drams = [nc.dram_tensor(f"a{g}", [128, N_KCH * R], BF16,
                              kind="ExternalInput") for g in range(2)]
    xts = [nc.dram_tensor(f"xt{i}", [128, IN_F // 4], BF16,
                          kind="ExternalInput") for i in range(4)]
    b_drams = [nc.dram_tensor(f"b{g}", [R, OUT_PG], BF16,
                              kind="ExternalInput") for g in range(2)]
    out = nc.dram_tensor("out", [TOK, N_OUT], BF16, kind="ExternalOutput")

    with TileContext(nc) as tc:
        with (
            tc.tile_pool(name="wpool", bufs=1) as wp,
            tc.tile_pool(name="xpool", bufs=1) as xp,
            tc.tile_pool(name="ps1", bufs=1, space="PSUM") as pp1,
            tc.tile_pool(name="ps2", bufs=psum_bufs, space="PSUM") as pp2,
            tc.tile_pool(name="stag", bufs=stag_bufs) as sp,
        ):
            # PE clock warmup: HAM throttles the PE to 1.2 GHz until it has
            # been busy ~3.4us.  Burn the load-phase idle time on fat zero
            # matmuls so real matmuls start at 2.4 GHz.
            wz = wp.tile([128, 512], BF16, tag="wz")
            nc.gpsimd.memset(wz[:], 0.0)
            wps = pp1.tile([128, 512], F32, tag="wps")
            for _ in range(n_warmup):
                nc.tensor.matmul(wps[:], lhsT=wz[:, 0:128], rhs=wz[:],
                                 start=True, stop=True)

            # Loads: two HWDGE rings (sync / scalar), A first (needed first),
            # x interleaved, B last (needed only at stage 2).
            a_sbs = []
            for g in range(2):
                atl = xp.tile([128, N_KCH * R], BF16, name=f"a{g}", tag=f"a{g}")
                a_sbs.append(atl)
            x_tiles = [xp.tile([128, IN_F // 4], BF16, name=f"x{i}", tag=f"x{i}")
                       for i in range(4)]
            b_sb = wp.tile([48, OUT_PG], BF16, tag="b")

            nc.sync.dma_start(out=a_sbs[0][:], in_=a_drams[0][:])
            nc.scalar.dma_start(out=a_sbs[1][:], in_=a_drams[1][:])
            nc.sync.dma_start(out=x_tiles[0][:], in_=xts[0][:])
            nc.scalar.dma_start(out=x_tiles[1][:], in_=xts[1][:])
            nc.sync.dma_start(out=x_tiles[2][:], in_=xts[2][:])
            nc.scalar.dma_start(out=x_tiles[3][:], in_=xts[3][:])
            nc.sync.dma_start(out=b_sb[0:R, :], in_=b_drams[0][:])
            nc.scalar.dma_start(out=b_sb[32:32 + R, :], in_=b_drams[1][:])

            # stage 1: t[0:16] += a0_n.T @ x_n, t[32:48] += a1_n.T @ x_n.
            # The two MMs of a chunk run concurrently in PE col-groups 0/1.
            tps = pp1.tile([48, TOK], F32)
            for n in range(N_KCH):
                xch = x_tiles[n // 8][:, (n % 8) * 128:(n % 8) * 128 + 128]
                nc.tensor.matmul(
                    tps[0:R, :],
                    lhsT=a_sbs[0][:, n * R:(n + 1) * R],
                    rhs=xch,
                    start=(n == 0), stop=(n == N_KCH - 1),
                )
                nc.tensor.matmul(
                    tps[32:32 + R, :],
                    lhsT=a_sbs[1][:, n * R:(n + 1) * R],
                    rhs=xch,
                    start=(n == 0), stop=(n == N_KCH - 1),
                )
            # t -> SBUF as bf16 (stage-2 stationary operand), per-group
            # slices so no unwritten PSUM rows are read.
            t_sb = wp.tile([48, TOK], BF16, tag="t")
            nc.vector.tensor_copy(t_sb[0:R, :], tps[0:R, :])
            nc.scalar.copy(t_sb[32:32 + R, :], tps[32:32 + R, :])

            # stage 2: per 512-col chunk j, concurrent row-tiled MM pair;
            # PSUM f32 -> bf16 staging via 3-engine round-robin cast copies;
            # one 256KB store per chunk covering both groups.
            n_ch = OUT_PG // 512            # 8 chunks per group
            per_store = n_ch // n_stores    # chunks per store (=1)
            # Only DVE and ACT can read PSUM (gpsimd has no PSUM access).
            cp_engines = [nc.vector.tensor_copy, nc.scalar.copy]
            st_engines = [nc.sync, nc.scalar]
            ci = 0
            for s in range(n_stores):
                stw = per_store * 512
                stg = sp.tile([TOK, 2 * stw], BF16, name="stg", tag="stg")
                for j2 in range(per_store):
                    j = s * per_store + j2
                    for g in (0, 1):
                        ps = pp2.tile([TOK, 512], F32)
                        nc.tensor.matmul(
                            ps[:],
                            lhsT=t_sb[32 * g:32 * g + R, :],
                            rhs=b_sb[32 * g:32 * g + R, j * 512:(j + 1) * 512],
                            start=True, stop=True,
                        )
                        dst_sl = stg[:, (g * per_store + j2) * 512:
                                     (g * per_store + j2 + 1) * 512]
                        cp_engines[ci % 2](dst_sl, ps[:])
                        ci += 1
                dst = out.rearrange("t (g o) -> t g o", g=2)[:, :, s * stw:(s + 1) * stw]
                src = stg.rearrange("t (g o) -> t g o", g=2)
                st_engines[s % 2].dma_start(out=dst, in_=src)
    nc.compile()
    return nc


def prep_weights(weight_A: np.ndarray, weight_B: np.ndarray):
    """Pack weights into PE layouts (replicated across cores), bf16."""
    weight_A = np.asarray(weight_A, np.float32)
    weight_B = np.asarray(weight_B, np.float32)
    # a{g}[p, n*R+m] = A_g[m, n*128+p]
    a_packs = []
    for g in range(2):
        Ag = weight_A[g * R:(g + 1) * R]                    # (16, 4096)
        a_packs.append(np.ascontiguousarray(
            Ag.reshape(R, N_KCH, 128).transpose(2, 1, 0)
        ).reshape(128, N_KCH * R).astype(NPBF16))
    b0 = np.ascontiguousarray(weight_B[0:OUT_PG].T).astype(NPBF16)
    b1 = np.ascontiguousarray(weight_B[OUT_PG:2 * OUT_PG].T).astype(NPBF16)
    return a_packs[0], a_packs[1], b0, b1


def prep_x_shard(xs: np.ndarray) -> np.ndarray:
    """(128, 4096) token shard -> transposed-tiled bf16 layout where
    tile[p, n*128+t] = xs[t, n*128+p] (contraction dim on partitions)."""
    return np.ascontiguousarray(
        xs.reshape(TOK, N_KCH, 128).transpose(2, 1, 0)
    ).reshape(128, IN_F).astype(NPBF16)


def make_in_maps(x: np.ndarray, weight_A: np.ndarray, weight_B: np.ndarray):
    xs_full = np.asarray(x, np.float32).reshape(N_CORES * TOK, IN_F)
    a0, a1, b0, b1 = prep_weights(weight_A, weight_B)
    in_maps = []
    for c in range(N_CORES):
        xt = prep_x_shard(xs_full[c * TOK:(c + 1) * TOK])
        m = {"a0": a0, "a1": a1, "b0": b0, "b1": b1}
        for i in range(4):
            m[f"xt{i}"] = np.ascontiguousarray(
                xt[:, i * (IN_F // 4):(i + 1) * (IN_F // 4)])
        in_maps.append(m)
    return in_maps


def assemble_output(results) -> np.ndarray:
    full = np.zeros((N_CORES * TOK, FULL_OUT), np.float32)
    for c in range(N_CORES):
        o = np.asarray(results[c]["out"]).astype(np.float32)
        full[c * TOK:(c + 1) * TOK, 0:OUT_PG] = o[:, 0:OUT_PG]
        full[c * TOK:(c + 1) * TOK, 2 * OUT_PG:3 * OUT_PG] = o[:, OUT_PG:2 * OUT_PG]
    return full.reshape(2, 512, FULL_OUT)


def run(x, weight_A, weight_B, **spmd_kwargs):
    key = "default"
    if key not in _NC_CACHE:
        _NC_CACHE[key] = build_nc()
    nc = _NC_CACHE[key]
    in_maps = make_in_maps(x, weight_A, weight_B)
    res = run_bass_kernel_spmd(nc, in_maps, list(range(N_CORES)), **spmd_kwargs)
    return assemble_output(res.results), res


def kernel(x, weight_A, weight_B):
    out, _ = run(x, weight_A, weight_B)
    return out
